# revision 16
# baseline (speedup 1.0000x reference)
"""AttnBlock (GroupNorm + single-head 1x1-conv attention + residual) on 8 TRN2 NeuronCores.

Data-parallel over batch (b=8): each core runs one full sample.
Per-core layout: x,y as [C=256, HW=4096], two 128-partition SBUF tiles each.

Precision plan (validated vs the fp32 reference, relmax ~1.5e-2):
  * GroupNorm stats, softmax normalization, residual: fp32.
  * q/k projections: f32r into fp8 e4m3 score operands.
  * scores, value path and Z: fp8 e4m3 DoubleRow matmuls (contraction 256
    per pass) — exp is emitted with an extra bias = -ln(16) so
    et = exp(s/16)/16 stays inside e4m3's ±240 range (the 1/16 factor
    cancels in U/Z).

Structure per core (b=1 sample):
  1. x,y DMA'd straight into f32r via gpsimd casting DMA. GroupNorm stats:
     per-partition sum (DVE reduce) + sum-of-squares (ACT Square with
     accum_out, squares scratched into the future q/k fp8 buffers),
     group reduction via tiny PE matmuls, rsqrt via sqrt+reciprocal+Newton.
  2. GroupNorm folded into the projections: q = (wq*s_x)@x + (wq@t_x + bq),
     likewise k with s_y/t_y; normalized activations never materialized.
  3. wp folded into the value path: vp[pix, m] = y^T-chunk @ (s_y*Wvp) + b,
     with Wvp = wv^T @ wp^T precomputed on host. The attention-value matmul
     directly accumulates the FINAL output channels — no separate
     output-projection stage, no PSUM->SBUF copies of U.
  4. Key tiles in pairs (kt0,kt1): 2 DoubleRow score matmuls fill a 2-bank
     PSUM tile [128,2,512]; ONE ACT exp instruction (1024 elems, scale=1/16,
     bias=-ln16) writes the fp8 pair tile et[128,2,512]; then 3 DoubleRow
     matmuls: U0/U1 (final channels) and Zbcast (ones row). Emission is
     software-pipelined at distance 2 pairs so PE never waits on ACT.
  5. out = U * (1/Z) + bp + x on DVE in fp32, DMA'd back per block.
"""

import os
import sys
import numpy as np

for _p in ("/opt/trn_rl_repo", "/root/.axon_site/_ro/trn_rl_repo"):
    if _p not in sys.path and os.path.isdir(_p):
        sys.path.append(_p)

import concourse.bass as bass
import concourse.tile as tile
from concourse import bacc, mybir
from concourse.bass import ts
from concourse.bass_utils import run_bass_kernel_spmd

F32 = mybir.dt.float32
F32R = mybir.dt.float32r
BF16 = mybir.dt.bfloat16
F8 = mybir.dt.float8e4
AX = mybir.AxisListType
OP = mybir.AluOpType
AF = mybir.ActivationFunctionType
PM = mybir.MatmulPerfMode

B = 8
C = 256
H = W = 64
HW = H * W          # 4096
P = 128             # partitions
NCT = C // P        # 2 channel tiles
NKT = HW // P       # 32 key tiles
NPAIR = NKT // 2    # 16 key-tile pairs
NQB = HW // 512     # 8 query blocks of 512
QB = 512
GSIZE = 64          # channels per group (4 groups of 64)
EPS = 1e-6
INV_N = 1.0 / (GSIZE * HW)
SM_SCALE = 1.0 / 16.0       # C ** -0.5
EXP_BIAS = -2.772588722239781  # -ln(16): keeps exp output within fp8 range

# vecs[:, col] layout (per-partition constants, one column pair per c-tile)
GAMMA, BETA, BQ, BK, BP, GIND = 0, 2, 4, 6, 8, 10


def _build_body(nc, tc, ctx, d):
    """Emit the per-sample kernel body. d: dict of dram tensor handles."""
    cp = ctx.enter_context(tc.tile_pool(name="const", bufs=1))
    sp = ctx.enter_context(tc.tile_pool(name="small", bufs=2))
    fin_pool = ctx.enter_context(tc.tile_pool(name="fin", bufs=2))
    et_pool = ctx.enter_context(tc.tile_pool(name="etp", bufs=4))
    xp = ctx.enter_context(tc.tile_pool(name="xdouble", bufs=2))
    sc2 = ctx.enter_context(tc.tile_pool(name="scratch", bufs=1))
    pa = ctx.enter_context(tc.tile_pool(name="pa", bufs=2, space="PSUM"))
    pu = ctx.enter_context(tc.tile_pool(name="pu", bufs=2, space="PSUM"))
    pz = ctx.enter_context(tc.tile_pool(name="pz", bufs=2, space="PSUM"))

    # ---- loads ----
    # x/y land directly in f32r via gpsimd casting DMA, chunked so stats can
    # start before the full tensor arrives. Small/constant loads ride HWDGE.
    def load_xy(name, dram, pool):
        tls = []
        for i in range(NCT):
            t = pool.tile([P, HW], F32R, tag=f"{name}{i}", name=f"{name}{i}")
            for j in range(2):
                sl = slice(j * HW // 2, (j + 1) * HW // 2)
                nc.gpsimd.dma_start(t[:, sl], dram.ap()[i * P:(i + 1) * P, sl])
            tls.append(t)
        return tls

    xr = load_xy("xr", d["x"], xp)
    yr = load_xy("yr", d["y"], cp)

    def load_w(name, dram):
        tls = []
        for i in range(NCT):
            t = cp.tile([P, C], F32, tag=f"{name}{i}", name=f"{name}{i}")
            nc.gpsimd.dma_start(t[:], dram.ap()[i * P:(i + 1) * P, :])
            tls.append(t)
        return tls

    wq_st = load_w("wqs", d["wqt"])
    wk_st = load_w("wks", d["wkt"])
    wvp_st = load_w("wvps", d["wvpr"])

    vecs = cp.tile([P, 12], F32, tag="vecs", name="vecs")
    nc.sync.dma_start(vecs[:], d["vecs"].ap()[:])
    gt_sb = cp.tile([2, P], F32, tag="gt", name="gt")
    nc.sync.dma_start(gt_sb[:], d["gt"].ap()[:])
    bvw_b = cp.tile([P, C], F32, tag="bvwb", name="bvwb")
    nc.sync.dma_start(bvw_b[:], d["bvw"].ap()[:])
    ones_f = cp.tile([P, P], F32, tag="onesf", name="onesf")
    nc.sync.dma_start(ones_f[:], d["ones"].ap()[:])
    ones_dr = cp.tile([P, 2, P], F8, tag="onesdr", name="onesdr")
    nc.vector.memset(ones_dr[:, 0, :], 1.0)
    nc.vector.memset(ones_dr[:, 1, :], 1.0)
    bias_t = cp.tile([P, 1], F32, tag="biast", name="biast")
    nc.vector.memset(bias_t[:], EXP_BIAS)

    qhp = cp.tile([P, 2, HW], F8, tag="qhp", name="qhp")
    khp = cp.tile([P, 2, HW], F8, tag="khp", name="khp")
    vpt = cp.tile([P, NPAIR, 2, C], F8, tag="vpt", name="vpt")

    # ---- group norm statistics -> per-channel scale/shift [P,1] per tile ----
    # Plain sums run as GPSIMD binary-tree adds: GPSIMD is idle mid-rep and
    # its queue reaches these once the (double-buffered) x / y data lands,
    # so in steady state the sums are ready before the rep boundary instead
    # of serializing behind the previous rep's DVE tail.
    def tree_sum(s32, out_col, nm):
        parts = sc2.tile([P, 2], F32, tag="rP", name=f"rP_{nm}")
        for c in range(2):
            base = c * 2048
            ta = sc2.tile([P, 1024], F32, tag="rA", name=f"rA_{nm}{c}")
            tb = sc2.tile([P, 512], F32, tag="rB", name=f"rB_{nm}{c}")
            nc.gpsimd.tensor_add(ta[:], s32[:, base:base + 1024],
                                 s32[:, base + 1024:base + 2048])
            bufs = [tb, ta]
            cur, w, i = ta, 512, 0
            while w >= 1:
                dst = parts[:, c:c + 1] if w == 1 else bufs[i % 2][:, 0:w]
                nc.gpsimd.tensor_add(dst, cur[:, 0:w], cur[:, w:2 * w])
                if w == 1:
                    break
                cur = bufs[i % 2]
                i += 1
                w //= 2
        nc.gpsimd.tensor_add(out_col, parts[:, 0:1], parts[:, 1:2])

    def gnorm_stats(src, scratch, tname):
        scales, shifts = [], []
        for ct in range(NCT):
            s32 = src[ct][:].bitcast(F32)
            stats = sp.tile([P, 2], F32, tag="stats", name=f"stats_{tname}{ct}")
            tree_sum(s32, stats[:, 0:1], f"{tname}{ct}")
            nc.scalar.activation(scratch[:, ct, :], s32, AF.Square,
                                 accum_out=stats[:, 1:2])
            gp = pa.tile([2, 2], F32, tag="a", name=f"gp_{tname}{ct}")
            nc.tensor.matmul(gp[:], vecs[:, GIND:GIND + 2], stats[:],
                             start=True, stop=True)
            st = sp.tile([2, 8], F32, tag="st", name=f"st_{tname}{ct}")
            nc.scalar.mul(st[:, 0:2], gp[:], INV_N)   # col0 mean, col1 E[x^2]
            nc.vector.tensor_mul(st[:, 2:3], st[:, 0:1], st[:, 0:1])   # mean^2
            nc.vector.tensor_sub(st[:, 3:4], st[:, 1:2], st[:, 2:3])   # var
            nc.vector.tensor_scalar_add(st[:, 7:8], st[:, 3:4], EPS)   # var+eps
            nc.scalar.activation(st[:, 4:5], st[:, 7:8], AF.Sqrt)
            nc.vector.reciprocal(st[:, 5:6], st[:, 4:5])               # r0
            # one Newton step: r = r0*(1.5 - 0.5*(var+eps)*r0^2)
            nc.vector.tensor_mul(st[:, 6:7], st[:, 5:6], st[:, 5:6])
            nc.vector.tensor_mul(st[:, 6:7], st[:, 7:8], st[:, 6:7])
            nc.vector.tensor_scalar(st[:, 6:7], st[:, 6:7], -0.5, 1.5,
                                    op0=OP.mult, op1=OP.add)
            nc.vector.tensor_mul(st[:, 5:6], st[:, 5:6], st[:, 6:7])   # rstd
            rps = pa.tile([P, 1], F32, tag="a", name=f"rps_{tname}{ct}")
            nc.tensor.matmul(rps[:], gt_sb[:], st[:, 5:6], start=True, stop=True)
            mps = pa.tile([P, 1], F32, tag="a", name=f"mps_{tname}{ct}")
            nc.tensor.matmul(mps[:], gt_sb[:], st[:, 0:1], start=True, stop=True)
            scale = sp.tile([P, 1], F32, tag=f"scale_{tname}{ct}",
                            name=f"scale_{tname}{ct}")
            nc.vector.tensor_mul(scale[:], rps[:], vecs[:, GAMMA + ct:GAMMA + ct + 1])
            shift = sp.tile([P, 1], F32, tag=f"shift_{tname}{ct}",
                            name=f"shift_{tname}{ct}")
            tmp = sp.tile([P, 1], F32, tag="gtmp", name=f"gtmp_{tname}{ct}")
            nc.vector.tensor_mul(tmp[:], mps[:], scale[:])
            nc.vector.tensor_sub(shift[:], vecs[:, BETA + ct:BETA + ct + 1], tmp[:])
            scales.append(scale)
            shifts.append(shift)
        return scales, shifts

    sc_x, sh_x = gnorm_stats(xr, qhp, "x")   # squares scratched into qhp
    sc_y, sh_y = gnorm_stats(yr, khp, "y")   # squares scratched into khp

    # ---- fold GroupNorm into projection weights ----
    # w' = wT * s[ci]  (per-partition);  b' = wT^T @ t + b  via tiny matmuls
    def prime_w(w_st, scales, wname):
        prim = []
        for ct in range(NCT):
            t = cp.tile([P, C], F32R, tag=f"{wname}{ct}", name=f"{wname}{ct}")
            nc.vector.tensor_scalar_mul(t[:], w_st[ct][:], scales[ct][:])
            prim.append(t)
        return prim

    wq_pr = prime_w(wq_st, sc_x, "wqp")
    wk_pr = prime_w(wk_st, sc_y, "wkp")
    wvp_pr = prime_w(wvp_st, sc_y, "wvpp")

    def bias_vec(w_st, shifts, bias_col, bname):
        bv = sp.tile([P, NCT], F32, tag=f"bv_{bname}", name=f"bv_{bname}")
        for m in range(NCT):
            ps = pa.tile([P, 1], F32, tag="a", name=f"bps_{bname}{m}")
            for ct in range(NCT):
                nc.tensor.matmul(ps[:], w_st[ct][:, ts(m, P)], shifts[ct][:],
                                 start=(ct == 0), stop=(ct == NCT - 1))
            nc.vector.tensor_add(bv[:, m:m + 1], ps[:],
                                 vecs[:, bias_col + m:bias_col + m + 1])
        return bv

    bq_v = bias_vec(wq_st, sh_x, BQ, "q")
    bk_v = bias_vec(wk_st, sh_y, BK, "k")

    # vp bias, broadcast over partitions: bvp2 = ones*t_y-chunks @ Wvp + bv@wpT
    bvp = pa.tile([P, C], F32, tag="a", name="bvp")
    for ct in range(NCT):
        tm = sp.tile([P, P], F32, tag="tmat", name=f"tmat{ct}")
        nc.vector.tensor_scalar_mul(tm[:], ones_f[:], sh_y[ct][:])
        nc.tensor.matmul(bvp[:], tm[:], wvp_st[ct][:],
                         start=(ct == 0), stop=(ct == NCT - 1))
    bvp2 = cp.tile([P, C], F32, tag="bvp2", name="bvp2")
    nc.vector.tensor_add(bvp2[:], bvp[:], bvw_b[:])

    # ---- projections q = wq'@x + bq', k = wk'@y + bk'  (fp8 pair layout) ----
    def proj(dst, w_pr, src, bv, pname):
        for m in range(NCT):
            for j in range(NQB):
                ps = pa.tile([P, QB], F32, tag="a", name=f"p_{pname}{m}_{j}")
                for ct in range(NCT):
                    nc.tensor.matmul(ps[:], w_pr[ct][:, ts(m, P)],
                                     src[ct][:, ts(j, QB)],
                                     start=(ct == 0), stop=(ct == NCT - 1))
                nc.vector.tensor_scalar_add(dst[:, m, ts(j, QB)], ps[:],
                                            bv[:, m:m + 1])

    proj(qhp, wq_pr, xr, bq_v, "q")
    proj(khp, wk_pr, yr, bk_v, "k")

    # ---- vp[pix, m] = y^T-chunk @ wvp' + bvp2, laid out in DoubleRow pairs ----
    for kt in range(NKT):
        ps = pa.tile([P, C], F32, tag="a", name=f"pv_{kt}")
        for ct in range(NCT):
            nc.tensor.matmul(ps[:], yr[ct][:, ts(kt, P)], wvp_pr[ct][:],
                             start=(ct == 0), stop=(ct == NCT - 1))
        nc.vector.tensor_add(vpt[:, kt // 2, kt % 2, :], ps[:], bvp2[:])

    # ---- attention, per 512-wide query block ----
    # Key tiles in pairs: 2 DoubleRow score matmuls -> [128,2,512] PSUM
    # (2 banks), one ACT exp instruction -> fp8 pair tile, then 3 DoubleRow
    # matmuls (U0/U1 final channels + Z broadcast). Emission pipelined at
    # distance 2 pairs so PE never sits on ACT's exp latency.
    out_ap = d["out"].ap()
    for qb in range(NQB):
        qsl = ts(qb, QB)
        u0 = pu.tile([P, QB], F32, tag="u", name=f"u0_{qb}")
        u1 = pu.tile([P, QB], F32, tag="u", name=f"u1_{qb}")
        zp = pz.tile([P, QB], F32, tag="z", name=f"z_{qb}")

        def uz(j, et):
            first, last = j == 0, j == NPAIR - 1
            nc.tensor.matmul(u0[:], vpt[:, j, :, 0:P], et[:],
                             start=first, stop=last, perf_mode=PM.DoubleRow)
            nc.tensor.matmul(u1[:], vpt[:, j, :, P:C], et[:],
                             start=first, stop=last, perf_mode=PM.DoubleRow)
            nc.tensor.matmul(zp[:], ones_dr[:], et[:],
                             start=first, stop=last, perf_mode=PM.DoubleRow)

        prev = prev2 = None
        for j in range(NPAIR):
            sps = pa.tile([P, 2, QB], F32, tag="a", name=f"s_{qb}_{j}")
            for half in range(2):
                kt = 2 * j + half
                nc.tensor.matmul(sps[:, half, :], khp[:, :, ts(kt, P)],
                                 qhp[:, :, qsl], start=True, stop=True,
                                 perf_mode=PM.DoubleRow)
            if prev2 is not None:
                uz(j - 2, prev2)
            prev2 = prev
            prev = et_pool.tile([P, 2, QB], F8, tag="et", name=f"et_{qb}_{j}")
            nc.scalar.activation(prev[:], sps[:], AF.Exp,
                                 bias=bias_t[:], scale=SM_SCALE)
        uz(NPAIR - 2, prev2)
        uz(NPAIR - 1, prev)

        zi = sp.tile([P, QB], F32, tag="zi", name=f"zi_{qb}")
        nc.vector.reciprocal_approx_fast(out=zi[:], in_=zp[:])
        for m, um in enumerate((u0, u1)):
            t1 = fin_pool.tile([P, QB], F32, tag="t1", name=f"t1_{qb}_{m}")
            nc.vector.tensor_mul(t1[:], um[:], zi[:])
            ot = fin_pool.tile([P, QB], F32, tag="ot", name=f"ot_{qb}_{m}")
            nc.vector.scalar_tensor_tensor(
                ot[:], t1[:], vecs[:, BP + m:BP + m + 1],
                xr[m][:, qsl].bitcast(F32), op0=OP.add, op1=OP.add)
            nc.sync.dma_start(out_ap[m * P:(m + 1) * P, qsl], ot[:])


def build_nc(rep=1):
    """Build + compile the single-core Bass program. rep>1 wraps the body in a
    dynamic loop (timing builds only)."""
    from contextlib import ExitStack
    nc = bacc.Bacc("TRN2", target_bir_lowering=False, debug=False,
                   enable_asserts=False, num_devices=B)
    d = {
        "x": nc.dram_tensor("x", (C, HW), F32, kind="ExternalInput"),
        "y": nc.dram_tensor("y", (C, HW), F32, kind="ExternalInput"),
        "wqt": nc.dram_tensor("wqt", (C, C), F32, kind="ExternalInput"),
        "wkt": nc.dram_tensor("wkt", (C, C), F32, kind="ExternalInput"),
        "wvpr": nc.dram_tensor("wvpr", (C, C), F32, kind="ExternalInput"),
        "vecs": nc.dram_tensor("vecs", (P, 12), F32, kind="ExternalInput"),
        "gt": nc.dram_tensor("gt", (2, P), F32, kind="ExternalInput"),
        "bvw": nc.dram_tensor("bvw", (P, C), F32, kind="ExternalInput"),
        "ones": nc.dram_tensor("ones", (P, P), F32, kind="ExternalInput"),
        "out": nc.dram_tensor("out", (C, HW), F32, kind="ExternalOutput"),
    }
    with tile.TileContext(nc) as tc:
        with ExitStack() as ctx:
            if rep > 1:
                with tc.For_i(0, rep, 1):
                    _build_body(nc, tc, ctx, d)
            else:
                _build_body(nc, tc, ctx, d)
    nc.compile()
    return nc


def make_in_maps(x, y, gn_gamma, gn_beta, wq, bq, wk, bk, wv, bv, wp, bp):
    """Host-side prep: per-core input dicts (core i gets sample i)."""
    f32 = np.float32

    def prep_w(w):
        return np.ascontiguousarray(np.asarray(w, f32).T)

    wqt, wkt = prep_w(wq), prep_w(wk)
    # Fused value/output-proj weight: vp = y^T @ (wv^T wp^T)  [ci, m]
    wvpr = np.ascontiguousarray(
        (np.asarray(wv, f32).T @ np.asarray(wp, f32).T))

    def cols(v):  # [C] -> [P, NCT] (column per c-tile)
        return np.asarray(v, f32).reshape(NCT, P).T

    vecs = np.zeros((P, 12), f32)
    vecs[:, GAMMA:GAMMA + 2] = cols(gn_gamma)
    vecs[:, BETA:BETA + 2] = cols(gn_beta)
    vecs[:, BQ:BQ + 2] = cols(bq)
    vecs[:, BK:BK + 2] = cols(bk)
    vecs[:, BP:BP + 2] = cols(bp)
    vecs[:GSIZE, GIND] = 1.0
    vecs[GSIZE:, GIND + 1] = 1.0
    gt = np.ascontiguousarray(vecs[:, GIND:GIND + 2].T)  # [2, P]
    bvw = np.tile((np.asarray(bv, f32) @ np.asarray(wp, f32).T)[None, :],
                  (P, 1))
    ones = np.ones((P, P), f32)

    xs = np.asarray(x, f32).reshape(B, C, HW)
    ys = np.asarray(y, f32).reshape(B, C, HW)
    shared = dict(wqt=wqt, wkt=wkt, wvpr=wvpr, vecs=vecs, gt=gt,
                  bvw=bvw, ones=ones)
    return [dict(x=np.ascontiguousarray(xs[i]), y=np.ascontiguousarray(ys[i]),
                 **shared) for i in range(B)]


_NC_CACHE = {}


def _get_nc(rep=1):
    if rep not in _NC_CACHE:
        _NC_CACHE[rep] = build_nc(rep)
    return _NC_CACHE[rep]


def run_on_cores(in_maps, rep=1):
    nc = _get_nc(rep)
    return run_bass_kernel_spmd(nc, in_maps, core_ids=list(range(len(in_maps))))


def kernel(**inputs):
    in_maps = make_in_maps(**inputs)
    res = run_on_cores(in_maps)
    out = np.stack([res.results[i]["out"].reshape(C, H, W) for i in range(B)])
    return out.astype(np.float32)


if __name__ == "__main__":
    rng = np.random.default_rng(0)
    ins = dict(
        x=rng.standard_normal((B, C, H, W), dtype=np.float32),
        y=rng.standard_normal((B, C, H, W), dtype=np.float32),
        gn_gamma=np.ones(C, np.float32), gn_beta=np.zeros(C, np.float32),
        wq=(rng.standard_normal((C, C)) / 16).astype(np.float32),
        bq=np.zeros(C, np.float32),
        wk=(rng.standard_normal((C, C)) / 16).astype(np.float32),
        bk=np.zeros(C, np.float32),
        wv=(rng.standard_normal((C, C)) / 16).astype(np.float32),
        bv=np.zeros(C, np.float32),
        wp=(rng.standard_normal((C, C)) / 16).astype(np.float32),
        bp=np.zeros(C, np.float32),
    )
    out = kernel(**ins)
    print("out", out.shape, out.dtype, np.abs(out).max())


# revision 17
# speedup vs baseline: 1.6099x; 1.6099x over previous
"""AttnBlock (GroupNorm + single-head 1x1-conv attention + residual) on 8 TRN2 NeuronCores.

Data-parallel over batch (b=8): each core runs one full sample.
Per-core layout: x,y as [C=256, HW=4096], two 128-partition SBUF tiles each.

Precision plan (validated vs the fp32 reference, relmax ~1.5e-2):
  * GroupNorm stats, softmax normalization, residual: fp32.
  * q/k projections: f32r into fp8 e4m3 score operands.
  * scores, value path and Z: fp8 e4m3 DoubleRow matmuls (contraction 256
    per pass) — exp is emitted with an extra bias = -ln(16) so
    et = exp(s/16)/16 stays inside e4m3's ±240 range (the 1/16 factor
    cancels in U/Z).

Structure per core (b=1 sample):
  1. x,y DMA'd straight into f32r via gpsimd casting DMA. GroupNorm stats:
     per-partition sum (DVE reduce) + sum-of-squares (ACT Square with
     accum_out, squares scratched into the future q/k fp8 buffers),
     group reduction via tiny PE matmuls, rsqrt via sqrt+reciprocal+Newton.
  2. GroupNorm folded into the projections: q = (wq*s_x)@x + (wq@t_x + bq),
     likewise k with s_y/t_y; normalized activations never materialized.
  3. wp folded into the value path: vp[pix, m] = y^T-chunk @ (s_y*Wvp) + b,
     with Wvp = wv^T @ wp^T precomputed on host. The attention-value matmul
     directly accumulates the FINAL output channels — no separate
     output-projection stage, no PSUM->SBUF copies of U.
  4. Key tiles in pairs (kt0,kt1): 2 DoubleRow score matmuls fill a 2-bank
     PSUM tile [128,2,512]; ONE ACT exp instruction (1024 elems, scale=1/16,
     bias=-ln16) writes the fp8 pair tile et[128,2,512]; then 3 DoubleRow
     matmuls: U0/U1 (final channels) and Zbcast (ones row). Emission is
     software-pipelined at distance 2 pairs so PE never waits on ACT.
  5. out = U * (1/Z) + bp + x on DVE in fp32, DMA'd back per block.
"""

import os
import sys
import numpy as np

for _p in ("/opt/trn_rl_repo", "/root/.axon_site/_ro/trn_rl_repo"):
    if _p not in sys.path and os.path.isdir(_p):
        sys.path.append(_p)

import concourse.bass as bass
import concourse.tile as tile
from concourse import bacc, mybir
from concourse.bass import ts
from concourse.bass_utils import run_bass_kernel_spmd

F32 = mybir.dt.float32
F32R = mybir.dt.float32r
BF16 = mybir.dt.bfloat16
F8 = mybir.dt.float8e4
AX = mybir.AxisListType
OP = mybir.AluOpType
AF = mybir.ActivationFunctionType
PM = mybir.MatmulPerfMode

B = 8
C = 256
H = W = 64
HW = H * W          # 4096
P = 128             # partitions
NCT = C // P        # 2 channel tiles
NKT = HW // P       # 32 key tiles
NPAIR = NKT // 2    # 16 key-tile pairs
NQB = HW // 512     # 8 query blocks of 512
QB = 512
GSIZE = 64          # channels per group (4 groups of 64)
EPS = 1e-6
INV_N = 1.0 / (GSIZE * HW)
SM_SCALE = 1.0 / 16.0       # C ** -0.5
EXP_BIAS = -2.772588722239781  # -ln(16): keeps exp output within fp8 range

# vecs[:, col] layout (per-partition constants, one column pair per c-tile)
GAMMA, BETA, BQ, BK, BP, GIND = 0, 2, 4, 6, 8, 10


def _build_body(nc, tc, ctx, d):
    """Emit the per-sample kernel body. d: dict of dram tensor handles."""
    cp = ctx.enter_context(tc.tile_pool(name="const", bufs=1))
    sp = ctx.enter_context(tc.tile_pool(name="small", bufs=2))
    fin_pool = ctx.enter_context(tc.tile_pool(name="fin", bufs=2))
    et_pool = ctx.enter_context(tc.tile_pool(name="etp", bufs=4))
    pa = ctx.enter_context(tc.tile_pool(name="pa", bufs=2, space="PSUM"))
    pu = ctx.enter_context(tc.tile_pool(name="pu", bufs=2, space="PSUM"))
    pz = ctx.enter_context(tc.tile_pool(name="pz", bufs=2, space="PSUM"))

    # ---- loads ----
    # x/y land directly in f32r via gpsimd casting DMA, chunked so stats can
    # start before the full tensor arrives. Small/constant loads ride HWDGE.
    def load_xy(name, dram):
        tls = []
        for i in range(NCT):
            t = cp.tile([P, HW], F32R, tag=f"{name}{i}", name=f"{name}{i}")
            for j in range(2):
                sl = slice(j * HW // 2, (j + 1) * HW // 2)
                nc.gpsimd.dma_start(t[:, sl], dram.ap()[i * P:(i + 1) * P, sl])
            tls.append(t)
        return tls

    xr = load_xy("xr", d["x"])
    yr = load_xy("yr", d["y"])

    def load_w(name, dram):
        tls = []
        for i in range(NCT):
            t = cp.tile([P, C], F32, tag=f"{name}{i}", name=f"{name}{i}")
            nc.gpsimd.dma_start(t[:], dram.ap()[i * P:(i + 1) * P, :])
            tls.append(t)
        return tls

    wq_st = load_w("wqs", d["wqt"])
    wk_st = load_w("wks", d["wkt"])
    wvp_st = load_w("wvps", d["wvpr"])

    vecs = cp.tile([P, 12], F32, tag="vecs", name="vecs")
    nc.sync.dma_start(vecs[:], d["vecs"].ap()[:])
    gt_sb = cp.tile([2, P], F32, tag="gt", name="gt")
    nc.sync.dma_start(gt_sb[:], d["gt"].ap()[:])
    bvw_b = cp.tile([P, C], F32, tag="bvwb", name="bvwb")
    nc.sync.dma_start(bvw_b[:], d["bvw"].ap()[:])
    ones_f = cp.tile([P, P], F32, tag="onesf", name="onesf")
    nc.sync.dma_start(ones_f[:], d["ones"].ap()[:])
    ones_dr = cp.tile([P, 2, P], F8, tag="onesdr", name="onesdr")
    nc.vector.memset(ones_dr[:, 0, :], 1.0)
    nc.vector.memset(ones_dr[:, 1, :], 1.0)
    bias_t = cp.tile([P, 1], F32, tag="biast", name="biast")
    nc.vector.memset(bias_t[:], EXP_BIAS)

    qhp = cp.tile([P, 2, HW], F8, tag="qhp", name="qhp")
    khp = cp.tile([P, 2, HW], F8, tag="khp", name="khp")
    vpt = cp.tile([P, NPAIR, 2, C], F8, tag="vpt", name="vpt")

    # ---- group norm statistics -> per-channel scale/shift [P,1] per tile ----
    def gnorm_stats(src, scratch, tname):
        scales, shifts = [], []
        for ct in range(NCT):
            s32 = src[ct][:].bitcast(F32)
            stats = sp.tile([P, 2], F32, tag="stats", name=f"stats_{tname}{ct}")
            nc.vector.reduce_sum(stats[:, 0:1], s32, axis=AX.X)
            nc.scalar.activation(scratch[:, ct, :], s32, AF.Square,
                                 accum_out=stats[:, 1:2])
            gp = pa.tile([2, 2], F32, tag="a", name=f"gp_{tname}{ct}")
            nc.tensor.matmul(gp[:], vecs[:, GIND:GIND + 2], stats[:],
                             start=True, stop=True)
            st = sp.tile([2, 8], F32, tag="st", name=f"st_{tname}{ct}")
            nc.scalar.mul(st[:, 0:2], gp[:], INV_N)   # col0 mean, col1 E[x^2]
            nc.vector.tensor_mul(st[:, 2:3], st[:, 0:1], st[:, 0:1])   # mean^2
            nc.vector.tensor_sub(st[:, 3:4], st[:, 1:2], st[:, 2:3])   # var
            nc.vector.tensor_scalar_add(st[:, 7:8], st[:, 3:4], EPS)   # var+eps
            nc.scalar.activation(st[:, 4:5], st[:, 7:8], AF.Sqrt)
            nc.vector.reciprocal(st[:, 5:6], st[:, 4:5])               # r0
            # one Newton step: r = r0*(1.5 - 0.5*(var+eps)*r0^2)
            nc.vector.tensor_mul(st[:, 6:7], st[:, 5:6], st[:, 5:6])
            nc.vector.tensor_mul(st[:, 6:7], st[:, 7:8], st[:, 6:7])
            nc.vector.tensor_scalar(st[:, 6:7], st[:, 6:7], -0.5, 1.5,
                                    op0=OP.mult, op1=OP.add)
            nc.vector.tensor_mul(st[:, 5:6], st[:, 5:6], st[:, 6:7])   # rstd
            rps = pa.tile([P, 1], F32, tag="a", name=f"rps_{tname}{ct}")
            nc.tensor.matmul(rps[:], gt_sb[:], st[:, 5:6], start=True, stop=True)
            mps = pa.tile([P, 1], F32, tag="a", name=f"mps_{tname}{ct}")
            nc.tensor.matmul(mps[:], gt_sb[:], st[:, 0:1], start=True, stop=True)
            scale = sp.tile([P, 1], F32, tag=f"scale_{tname}{ct}",
                            name=f"scale_{tname}{ct}")
            nc.vector.tensor_mul(scale[:], rps[:], vecs[:, GAMMA + ct:GAMMA + ct + 1])
            shift = sp.tile([P, 1], F32, tag=f"shift_{tname}{ct}",
                            name=f"shift_{tname}{ct}")
            tmp = sp.tile([P, 1], F32, tag="gtmp", name=f"gtmp_{tname}{ct}")
            nc.vector.tensor_mul(tmp[:], mps[:], scale[:])
            nc.vector.tensor_sub(shift[:], vecs[:, BETA + ct:BETA + ct + 1], tmp[:])
            scales.append(scale)
            shifts.append(shift)
        return scales, shifts

    sc_x, sh_x = gnorm_stats(xr, qhp, "x")   # squares scratched into qhp
    sc_y, sh_y = gnorm_stats(yr, khp, "y")   # squares scratched into khp

    # ---- fold GroupNorm into projection weights ----
    # w' = wT * s[ci]  (per-partition);  b' = wT^T @ t + b  via tiny matmuls
    def prime_w(w_st, scales, wname):
        prim = []
        for ct in range(NCT):
            t = cp.tile([P, C], F32R, tag=f"{wname}{ct}", name=f"{wname}{ct}")
            nc.vector.tensor_scalar_mul(t[:], w_st[ct][:], scales[ct][:])
            prim.append(t)
        return prim

    wq_pr = prime_w(wq_st, sc_x, "wqp")
    wk_pr = prime_w(wk_st, sc_y, "wkp")
    wvp_pr = prime_w(wvp_st, sc_y, "wvpp")

    def bias_vec(w_st, shifts, bias_col, bname):
        bv = sp.tile([P, NCT], F32, tag=f"bv_{bname}", name=f"bv_{bname}")
        for m in range(NCT):
            ps = pa.tile([P, 1], F32, tag="a", name=f"bps_{bname}{m}")
            for ct in range(NCT):
                nc.tensor.matmul(ps[:], w_st[ct][:, ts(m, P)], shifts[ct][:],
                                 start=(ct == 0), stop=(ct == NCT - 1))
            nc.vector.tensor_add(bv[:, m:m + 1], ps[:],
                                 vecs[:, bias_col + m:bias_col + m + 1])
        return bv

    bq_v = bias_vec(wq_st, sh_x, BQ, "q")
    bk_v = bias_vec(wk_st, sh_y, BK, "k")

    # vp bias, broadcast over partitions: bvp2 = ones*t_y-chunks @ Wvp + bv@wpT
    bvp = pa.tile([P, C], F32, tag="a", name="bvp")
    for ct in range(NCT):
        tm = sp.tile([P, P], F32, tag="tmat", name=f"tmat{ct}")
        nc.vector.tensor_scalar_mul(tm[:], ones_f[:], sh_y[ct][:])
        nc.tensor.matmul(bvp[:], tm[:], wvp_st[ct][:],
                         start=(ct == 0), stop=(ct == NCT - 1))
    bvp2 = cp.tile([P, C], F32, tag="bvp2", name="bvp2")
    nc.vector.tensor_add(bvp2[:], bvp[:], bvw_b[:])

    # ---- projections q = wq'@x + bq', k = wk'@y + bk'  (fp8 pair layout) ----
    def proj(dst, w_pr, src, bv, pname):
        for m in range(NCT):
            for j in range(NQB):
                ps = pa.tile([P, QB], F32, tag="a", name=f"p_{pname}{m}_{j}")
                for ct in range(NCT):
                    nc.tensor.matmul(ps[:], w_pr[ct][:, ts(m, P)],
                                     src[ct][:, ts(j, QB)],
                                     start=(ct == 0), stop=(ct == NCT - 1))
                nc.vector.tensor_scalar_add(dst[:, m, ts(j, QB)], ps[:],
                                            bv[:, m:m + 1])

    proj(qhp, wq_pr, xr, bq_v, "q")
    proj(khp, wk_pr, yr, bk_v, "k")

    # ---- vp[pix, m] = y^T-chunk @ wvp' + bvp2, laid out in DoubleRow pairs ----
    for kt in range(NKT):
        ps = pa.tile([P, C], F32, tag="a", name=f"pv_{kt}")
        for ct in range(NCT):
            nc.tensor.matmul(ps[:], yr[ct][:, ts(kt, P)], wvp_pr[ct][:],
                             start=(ct == 0), stop=(ct == NCT - 1))
        nc.vector.tensor_add(vpt[:, kt // 2, kt % 2, :], ps[:], bvp2[:])

    # ---- attention, per 512-wide query block ----
    # Key tiles in pairs: 2 DoubleRow score matmuls -> [128,2,512] PSUM
    # (2 banks), one ACT exp instruction -> fp8 pair tile, then 3 DoubleRow
    # matmuls (U0/U1 final channels + Z broadcast). Emission pipelined at
    # distance 2 pairs so PE never sits on ACT's exp latency.
    out_ap = d["out"].ap()
    for qb in range(NQB):
        qsl = ts(qb, QB)
        u0 = pu.tile([P, QB], F32, tag="u", name=f"u0_{qb}")
        u1 = pu.tile([P, QB], F32, tag="u", name=f"u1_{qb}")
        zp = pz.tile([P, QB], F32, tag="z", name=f"z_{qb}")

        def uz(j, et):
            first, last = j == 0, j == NPAIR - 1
            nc.tensor.matmul(u0[:], vpt[:, j, :, 0:P], et[:],
                             start=first, stop=last, perf_mode=PM.DoubleRow)
            nc.tensor.matmul(u1[:], vpt[:, j, :, P:C], et[:],
                             start=first, stop=last, perf_mode=PM.DoubleRow)
            nc.tensor.matmul(zp[:], ones_dr[:], et[:],
                             start=first, stop=last, perf_mode=PM.DoubleRow)

        prev = prev2 = None
        for j in range(NPAIR):
            sps = pa.tile([P, 2, QB], F32, tag="a", name=f"s_{qb}_{j}")
            for half in range(2):
                kt = 2 * j + half
                nc.tensor.matmul(sps[:, half, :], khp[:, :, ts(kt, P)],
                                 qhp[:, :, qsl], start=True, stop=True,
                                 perf_mode=PM.DoubleRow)
            if prev2 is not None:
                uz(j - 2, prev2)
            prev2 = prev
            prev = et_pool.tile([P, 2, QB], F8, tag="et", name=f"et_{qb}_{j}")
            nc.scalar.activation(prev[:], sps[:], AF.Exp,
                                 bias=bias_t[:], scale=SM_SCALE)
        uz(NPAIR - 2, prev2)
        uz(NPAIR - 1, prev)

        zi = sp.tile([P, QB], F32, tag="zi", name=f"zi_{qb}")
        nc.vector.reciprocal_approx_fast(out=zi[:], in_=zp[:])
        for m, um in enumerate((u0, u1)):
            t1 = fin_pool.tile([P, QB], F32, tag="t1", name=f"t1_{qb}_{m}")
            nc.vector.tensor_mul(t1[:], um[:], zi[:])
            ot = fin_pool.tile([P, QB], F32, tag="ot", name=f"ot_{qb}_{m}")
            nc.vector.scalar_tensor_tensor(
                ot[:], t1[:], vecs[:, BP + m:BP + m + 1],
                xr[m][:, qsl].bitcast(F32), op0=OP.add, op1=OP.add)
            nc.gpsimd.dma_start(out_ap[m * P:(m + 1) * P, qsl], ot[:])


def build_nc(rep=1):
    """Build + compile the single-core Bass program. rep>1 wraps the body in a
    dynamic loop (timing builds only)."""
    from contextlib import ExitStack
    nc = bacc.Bacc("TRN2", target_bir_lowering=False, debug=False,
                   enable_asserts=False, num_devices=B)
    d = {
        "x": nc.dram_tensor("x", (C, HW), F32, kind="ExternalInput"),
        "y": nc.dram_tensor("y", (C, HW), F32, kind="ExternalInput"),
        "wqt": nc.dram_tensor("wqt", (C, C), F32, kind="ExternalInput"),
        "wkt": nc.dram_tensor("wkt", (C, C), F32, kind="ExternalInput"),
        "wvpr": nc.dram_tensor("wvpr", (C, C), F32, kind="ExternalInput"),
        "vecs": nc.dram_tensor("vecs", (P, 12), F32, kind="ExternalInput"),
        "gt": nc.dram_tensor("gt", (2, P), F32, kind="ExternalInput"),
        "bvw": nc.dram_tensor("bvw", (P, C), F32, kind="ExternalInput"),
        "ones": nc.dram_tensor("ones", (P, P), F32, kind="ExternalInput"),
        "out": nc.dram_tensor("out", (C, HW), F32, kind="ExternalOutput"),
    }
    with tile.TileContext(nc) as tc:
        with ExitStack() as ctx:
            if rep > 1:
                with tc.For_i(0, rep, 1):
                    _build_body(nc, tc, ctx, d)
            else:
                _build_body(nc, tc, ctx, d)
    nc.compile()
    return nc


def make_in_maps(x, y, gn_gamma, gn_beta, wq, bq, wk, bk, wv, bv, wp, bp):
    """Host-side prep: per-core input dicts (core i gets sample i)."""
    f32 = np.float32

    def prep_w(w):
        return np.ascontiguousarray(np.asarray(w, f32).T)

    wqt, wkt = prep_w(wq), prep_w(wk)
    # Fused value/output-proj weight: vp = y^T @ (wv^T wp^T)  [ci, m]
    wvpr = np.ascontiguousarray(
        (np.asarray(wv, f32).T @ np.asarray(wp, f32).T))

    def cols(v):  # [C] -> [P, NCT] (column per c-tile)
        return np.asarray(v, f32).reshape(NCT, P).T

    vecs = np.zeros((P, 12), f32)
    vecs[:, GAMMA:GAMMA + 2] = cols(gn_gamma)
    vecs[:, BETA:BETA + 2] = cols(gn_beta)
    vecs[:, BQ:BQ + 2] = cols(bq)
    vecs[:, BK:BK + 2] = cols(bk)
    vecs[:, BP:BP + 2] = cols(bp)
    vecs[:GSIZE, GIND] = 1.0
    vecs[GSIZE:, GIND + 1] = 1.0
    gt = np.ascontiguousarray(vecs[:, GIND:GIND + 2].T)  # [2, P]
    bvw = np.tile((np.asarray(bv, f32) @ np.asarray(wp, f32).T)[None, :],
                  (P, 1))
    ones = np.ones((P, P), f32)

    xs = np.asarray(x, f32).reshape(B, C, HW)
    ys = np.asarray(y, f32).reshape(B, C, HW)
    shared = dict(wqt=wqt, wkt=wkt, wvpr=wvpr, vecs=vecs, gt=gt,
                  bvw=bvw, ones=ones)
    return [dict(x=np.ascontiguousarray(xs[i]), y=np.ascontiguousarray(ys[i]),
                 **shared) for i in range(B)]


_NC_CACHE = {}


def _get_nc(rep=1):
    if rep not in _NC_CACHE:
        _NC_CACHE[rep] = build_nc(rep)
    return _NC_CACHE[rep]


def run_on_cores(in_maps, rep=1):
    nc = _get_nc(rep)
    return run_bass_kernel_spmd(nc, in_maps, core_ids=list(range(len(in_maps))))


def kernel(**inputs):
    in_maps = make_in_maps(**inputs)
    res = run_on_cores(in_maps)
    out = np.stack([res.results[i]["out"].reshape(C, H, W) for i in range(B)])
    return out.astype(np.float32)


if __name__ == "__main__":
    rng = np.random.default_rng(0)
    ins = dict(
        x=rng.standard_normal((B, C, H, W), dtype=np.float32),
        y=rng.standard_normal((B, C, H, W), dtype=np.float32),
        gn_gamma=np.ones(C, np.float32), gn_beta=np.zeros(C, np.float32),
        wq=(rng.standard_normal((C, C)) / 16).astype(np.float32),
        bq=np.zeros(C, np.float32),
        wk=(rng.standard_normal((C, C)) / 16).astype(np.float32),
        bk=np.zeros(C, np.float32),
        wv=(rng.standard_normal((C, C)) / 16).astype(np.float32),
        bv=np.zeros(C, np.float32),
        wp=(rng.standard_normal((C, C)) / 16).astype(np.float32),
        bp=np.zeros(C, np.float32),
    )
    out = kernel(**ins)
    print("out", out.shape, out.dtype, np.abs(out).max())
